# revision 8
# baseline (speedup 1.0000x reference)
"""GCN (2-layer + mean-pool + classifier) Bass/Tile kernel for 8 Trainium2
NeuronCores, self-contained.  v2 redesign.

Sharding: dst-node partitioning (12544 nodes / 98 windows of 128 per core).

Layer 1: host stages dense dst-sorted message payloads (u1 = x*dinv rows) in
32-dst "slots" (4 per window); scatter via NARROW [128 msg, 32 dst] fp8
one-hot matmuls targeting PSUM quadrants (tile_position), accumulating chunks
per slot.  Chunk structure is the max across cores so the shared SPMD program
fits every core (shorter cores pad with zero rows).  fin1 computes
v2 = (dinv*relu(y1 W1 + b1)) @ W2 per node (W2 folded in before the layer-2
aggregation, with which it commutes).

v2 is AllGathered in two halves; layer 2 gathers v2 messages with the
bucketed one-hot scheme (16-window groups x 8-rank buckets, fp8 one-hot
stationaries), writes gather output straight to an SBUF table, and the
(b, I_lo)-regroup "permute" runs as ONE SBUF->SBUF DMA per window (no DRAM
round-trip).  Scatter uses fat [128,128] fp8 one-hots; overflow messages ride
batched indirect row-gathers from v2_dram.  fin2 applies dinv/bias/relu
rowwise (no transposes).  Pooling: one matmul per window with h2row as the
32-col stationary and a [128 node, 512 graph] fp8 one-hot as the moving
operand, accumulating into a persistent [32, 512] PSUM tile; the head then
runs transpose-free.  Output [512, 2] f32 (identical on every core).
"""
import numpy as np
import ml_dtypes

# ---------------------------------------------------------------- constants
N = 100000
N_PAD = 100352
NBLK = 784                     # 128-node src blocks
NCORES = 8
NPC = N_PAD // NCORES          # nodes per core = 12544
WPC = NPC // 128               # windows per core = 98
WH = WPC // 2                  # windows per allgather half = 49
NWG = 6                        # window groups of 16 (windows 0..95)
B = 8                          # bucket slots per (window, block)
NGT = NBLK * NWG               # L2 gather tiles
NST = WPC * 49                 # L2 scatter tiles (49 per window)
SPILL_CAP = [3] * 96 + [36, 36]
NSPILL = sum(SPILL_CAP)
SPILL_T0 = np.concatenate([[0], np.cumsum(SPILL_CAP)]).astype(int)
NGRAPH = 512
FP8NP = ml_dtypes.float8_e4m3
BF16NP = ml_dtypes.bfloat16

_LAST_RESULTS = None


# ------------------------------------------------------------------ patches
def _install_patches():
    import json

    import concourse.mybir as mybir
    import concourse.tile as tile_mod
    from concourse.vector_clock import ScopedClock

    if not getattr(tile_mod.TileContext, "_gcn_patched", False):
        def _drain_and_barrier(self, tick_clock, wait_clock):
            nc = self.nc
            drain_inst = nc.sync.drain()
            wait_clock.add_sem_waits(
                drain_inst.ins, ScopedClock({None: tick_clock.global_clock}))
            si = drain_inst.ins.sync_info
            waits = list(si.on_wait) if si is not None and si.on_wait else []
            if len(waits) > 1:
                si.on_wait = waits[:1]
                for w in waits[1:]:
                    extra = nc.sync.drain()
                    extra.ins.sync_info = mybir.SyncInfo(
                        on_wait=[w], on_update=[])
            nc.all_engine_barrier()
            assert self.sems is not None
            popped = nc._tile_sem_poison_stack.pop()
            assert popped is self._sem_poison
            nc.clear_and_free_semaphores(list(self.sems.allocated().values()))
            nc.all_engine_barrier()

        tile_mod.TileContext._drain_and_barrier = _drain_and_barrier
        tile_mod.TileContext._gcn_patched = True

    import concourse.bass as bass_mod

    if not getattr(bass_mod.Bass, "_wait_split_patched", False):
        orig = bass_mod.Bass.to_json_bytes

        def _split(data):
            j = json.loads(data)
            cnt = [0]

            def fix(insts):
                out = []
                for inst in insts:
                    si = inst.get("sync_info")
                    waits = si.get("on_wait") if si else None
                    if waits and len(waits) > 1:
                        for w in waits[:-1]:
                            cnt[0] += 1
                            out.append({
                                "debug": inst.get("debug", 0),
                                "engine": inst["engine"],
                                "ins": [], "outs": [],
                                "name": f"WSPL-{cnt[0]}-{inst['name']}",
                                "opcode": "EventSemaphore",
                                "sync_info": {"on_update": [], "on_wait": [w]},
                            })
                        si["on_wait"] = [waits[-1]]
                    out.append(inst)
                insts[:] = out

            def walk(d):
                if isinstance(d, dict):
                    for k, v in d.items():
                        if k == "instructions" and isinstance(v, list):
                            fix(v)
                        else:
                            walk(v)
                elif isinstance(d, list):
                    for e in d:
                        walk(e)

            walk(j)
            return json.dumps(j).encode()

        def to_json_bytes(self, *a, **kw):
            return _split(orig(self, *a, **kw))

        bass_mod.Bass.to_json_bytes = to_json_bytes
        bass_mod.Bass._wait_split_patched = True


# ----------------------------------------------------------------- cpu prep
def _prepare(x, edge_index, batch, W1, b1, W2, b2, Wc, bc):
    src = np.asarray(edge_index[0], dtype=np.int64)
    dst = np.asarray(edge_index[1], dtype=np.int64)
    batch = np.asarray(batch, dtype=np.int64)
    x = np.asarray(x, dtype=np.float32)

    deg = np.ones(N_PAD, dtype=np.float32)
    np.add.at(deg, dst, 1.0)
    dinv = (1.0 / np.sqrt(deg)).astype(np.float32)

    u1_rows = np.zeros((N_PAD, 8), dtype=np.float32)
    u1_rows[:N] = x
    u1_rows *= dinv[:, None]
    u1_rows = u1_rows.astype(BF16NP)

    # edge list incl. self-loops, sorted by dst
    loop = np.arange(N_PAD, dtype=np.int64)
    s_all = np.concatenate([src, loop])
    d_all = np.concatenate([dst, loop])
    order = np.argsort(d_all, kind="stable")
    s_all, d_all = s_all[order], d_all[order]

    win = (d_all // 128).astype(np.int64)
    wcnt = np.bincount(win, minlength=N_PAD // 128)
    wstart = np.concatenate([[0], np.cumsum(wcnt)])

    eye_bf16 = np.eye(128, dtype=np.float32).astype(BF16NP)

    cnt = np.zeros(NGRAPH, dtype=np.float32)
    np.add.at(cnt, batch, 1.0)
    cnt_inv = np.where(cnt > 0, 1.0 / np.maximum(cnt, 1.0), 1.0).astype(np.float32)
    cntbc = np.broadcast_to(cnt_inv[None, :], (2, NGRAPH)).copy()

    # ---- L1 chunk structure: per (window, slot), max chunk count over cores
    # slot_cnt[c, w, s] = messages with dst window (c,w), dst%128 in 32-slot s
    slot_key = (d_all // 32).astype(np.int64)       # global 32-dst slot id
    scnt = np.bincount(slot_key, minlength=N_PAD // 32)
    scnt = scnt.reshape(NCORES, WPC, 4)
    nch_slot = (np.ceil(scnt.max(axis=0) / 128).astype(int))  # [WPC, 4]
    assert nch_slot.min() >= 1
    chunks1 = []            # per window: list of (slot, first, last)
    ch_off1 = [0]
    tot = 0
    for w in range(WPC):
        wch = []
        for s in range(4):
            k = int(nch_slot[w, s])
            for j in range(k):
                wch.append((s, j == 0, j == k - 1))
            tot += k
        chunks1.append(wch)
        ch_off1.append(tot)
    nch1 = tot

    per_core = []
    for c in range(NCORES):
        base = c * NPC
        # ---- layer 1 payloads + narrow one-hots (shared structure)
        st1 = np.zeros((128, nch1, 8), dtype=BF16NP)
        s_dst1 = np.zeros((128, nch1, 32), dtype=FP8NP)
        for w in range(WPC):
            gw = c * WPC + w
            lo, hi = int(wstart[gw]), int(wstart[gw + 1])
            dd = (d_all[lo:hi] % 128).astype(np.int64)
            ss = s_all[lo:hi]
            col = ch_off1[w]
            for s in range(4):
                idx = np.where((dd // 32) == s)[0]
                n = len(idx)
                k = int(nch_slot[w, s])
                assert n <= k * 128
                for j in range(k):
                    seg = idx[j * 128:(j + 1) * 128]
                    if len(seg):
                        q = np.arange(len(seg))
                        st1[q, col, :] = u1_rows[ss[seg]]
                        s_dst1[q, col, dd[seg] % 32] = 1.0
                    col += 1

        # ---- layer 2: bucketed one-hot gather structures (baseline scheme)
        lo, hi = int(wstart[c * WPC]), int(wstart[(c + 1) * WPC])
        es, ed = s_all[lo:hi], d_all[lo:hi]
        w_ = (ed // 128) - c * WPC
        I = es // 128
        rs = es % 128
        rd = ed % 128

        o2 = np.lexsort((I, w_))
        w2_, I2, rs2, rd2 = w_[o2], I[o2], rs[o2], rd[o2]
        key = w2_ * NBLK + I2
        _, start, cnts = np.unique(key, return_index=True, return_counts=True)
        rank = np.arange(len(key)) - np.repeat(start, cnts)

        main = (w2_ < 96) & (rank < B)
        spm = ~main

        wm, Im, rsm, rdm, bm = (a[main] for a in (w2_, I2, rs2, rd2, rank))
        g = wm // 16
        w_lo = wm % 16
        I_lo, I_hi = Im % 16, Im // 16
        gt = g * NBLK + (I_lo * 49 + I_hi)
        slot = w_lo * 8 + bm
        s_src = np.zeros((128, NGT, 128), dtype=FP8NP)
        s_src[rsm, gt, slot] = 1.0
        st = wm * 49 + I_hi
        sp_part = bm * 16 + I_lo
        s_dst2 = np.zeros((128, NST, 128), dtype=FP8NP)
        s_dst2[sp_part, st, rdm] = 1.0

        ws, Is, rss, rds = (a[spm] for a in (w2_, I2, rs2, rd2))
        o3 = np.argsort(ws, kind="stable")
        ws, Is, rss, rds = ws[o3], Is[o3], rss[o3], rds[o3]
        wstart2 = np.searchsorted(ws, np.arange(WPC + 1))
        spill_idx = np.zeros((128, NSPILL), dtype=np.int32)
        s_spill = np.zeros((128, NSPILL, 128), dtype=FP8NP)
        for wi in range(WPC):
            a, bnd = wstart2[wi], wstart2[wi + 1]
            nsp = bnd - a
            if nsp > SPILL_CAP[wi] * 128:
                raise RuntimeError(
                    f"core {c} window {wi}: spill {nsp} > {SPILL_CAP[wi]*128}")
            k = np.arange(nsp)
            t = SPILL_T0[wi] + k // 128
            p = k % 128
            spill_idx[p, t] = (rss[a:bnd] * NBLK + Is[a:bnd]).astype(np.int32)
            s_spill[p, t, rds[a:bnd]] = 1.0

        dinv_col = np.ascontiguousarray(
            dinv[base:base + NPC].reshape(WPC, 128).T).copy()  # [128, WPC]
        dinv32 = np.broadcast_to(
            dinv[base:base + NPC].reshape(WPC, 128)[None, :, :],
            (32, WPC, 128)).astype(BF16NP).copy()              # [32, WPC, 128]
        b2bc = np.broadcast_to(
            np.asarray(b2, np.float32)[None, :], (128, 32)).copy()

        # ---- pooling: fat-moving [128 node, 512 graph] one-hot per window
        s_pool = np.zeros((128, WPC, NGRAPH), dtype=FP8NP)
        nodes = np.arange(base, base + NPC)
        valid = nodes < N
        gids = batch[np.minimum(nodes, N - 1)]
        wv = (nodes - base) // 128
        pv = (nodes - base) % 128
        s_pool[pv[valid], wv[valid], gids[valid]] = 1.0

        per_core.append({
            "st1": st1, "s_dst1": s_dst1,
            "s_src": s_src, "s_dst2": s_dst2,
            "s_spilldst": s_spill, "spill_idx": spill_idx,
            "dinv_col": dinv_col, "dinv32": dinv32, "b2bc": b2bc,
            "s_pool": s_pool,
            "w1": np.asarray(W1, np.float32).astype(BF16NP),
            "b1": np.asarray(b1, np.float32).reshape(32, 1).copy(),
            "w2": np.asarray(W2, np.float32).astype(BF16NP),
            "wc": np.asarray(Wc, np.float32).copy(),
            "bc": np.asarray(bc, np.float32).reshape(2, 1).copy(),
            "eye_bf16": eye_bf16,
            "cntbc": cntbc,
        })
    meta = {"nch1": nch1, "chunks1": chunks1, "ch_off1": ch_off1}
    return per_core, meta


# ------------------------------------------------------------------ builder
def _build_nc(meta):
    import concourse.bass as bass
    import concourse.mybir as mybir
    from concourse.tile import TileContext

    FP8 = mybir.dt.float8e4
    BF16 = mybir.dt.bfloat16
    F32 = mybir.dt.float32
    I32 = mybir.dt.int32
    AF = mybir.ActivationFunctionType

    NCH1 = meta["nch1"]
    CHUNKS1 = meta["chunks1"]
    CH_OFF1 = meta["ch_off1"]
    WB1 = 4  # L1 windows per load batch
    MAXB1 = max(CH_OFF1[min(w0 + WB1, WPC)] - CH_OFF1[w0]
                for w0 in range(0, WPC, WB1))

    nc = bass.Bass(target_bir_lowering=True)

    def inp(name, shape, dt):
        return nc.dram_tensor(name, shape, dt, kind="ExternalInput")

    st1 = inp("st1", [128, NCH1, 8], BF16)
    s_dst1 = inp("s_dst1", [128, NCH1, 32], FP8)
    s_src = inp("s_src", [128, NGT, 128], FP8)
    s_dst2 = inp("s_dst2", [128, NST, 128], FP8)
    s_spill = inp("s_spilldst", [128, NSPILL, 128], FP8)
    spill_idx = inp("spill_idx", [128, NSPILL], I32)
    dinv_col = inp("dinv_col", [128, WPC], F32)
    dinv32 = inp("dinv32", [32, WPC, 128], BF16)
    b2bc = inp("b2bc", [128, 32], F32)
    s_pool = inp("s_pool", [128, WPC, NGRAPH], FP8)
    w1 = inp("w1", [8, 32], BF16)
    b1 = inp("b1", [32, 1], F32)
    w2 = inp("w2", [32, 32], BF16)
    wc = inp("wc", [32, 2], F32)
    bc = inp("bc", [2, 1], F32)
    eye_bf16 = inp("eye_bf16", [128, 128], BF16)
    cntbc = inp("cntbc", [2, NGRAPH], F32)
    out = nc.dram_tensor("out", [NGRAPH, 2], F32, kind="ExternalOutput")

    v2_loc = [nc.dram_tensor(f"v2_loc{h}", [WH * 128, 32], BF16)
              for h in range(2)]
    v2_gh = [nc.dram_tensor(f"v2_gh{h}", [NCORES, WH * 128, 32], BF16)
             for h in range(2)]
    v2_dram = nc.dram_tensor("v2_dram", [128 * NBLK, 32], BF16)
    pool_in = nc.dram_tensor("pool_in", [32, NGRAPH], F32)
    pool_out = nc.dram_tensor("pool_out", [32, NGRAPH], F32)

    groups = [list(range(NCORES))]

    with TileContext(nc) as tc:
        with tc.tile_pool(name="glob", bufs=1) as gl, \
             tc.tile_pool(name="yps", bufs=2, space="PSUM") as yp, \
             tc.tile_pool(name="trps", bufs=1, space="PSUM") as tp, \
             tc.tile_pool(name="hps", bufs=2, space="PSUM") as hp, \
             tc.tile_pool(name="plps", bufs=1, space="PSUM") as plp, \
             tc.tile_pool(name="gbps", bufs=2, space="PSUM") as gbp:

            def load(t, src_ap, eng=None):
                (eng or nc.sync).dma_start(out=t[:], in_=src_ap[:])
                return t

            dinv_t = load(gl.tile([128, WPC], F32, tag="dinvc", name="dinvc"),
                          dinv_col)
            eyeb_t = load(gl.tile([128, 128], BF16, tag="eyeb", name="eyeb"),
                          eye_bf16)
            w1_t = load(gl.tile([8, 32], BF16, tag="w1t", name="w1t"), w1)
            b1_t = load(gl.tile([32, 1], F32, tag="b1t", name="b1t"), b1)
            w2_t = load(gl.tile([32, 32], BF16, tag="w2t", name="w2t"), w2)
            b2bc_t = load(gl.tile([128, 32], F32, tag="b2bct", name="b2bct"),
                          b2bc)
            wc_t = load(gl.tile([32, 2], F32, tag="wct", name="wct"), wc)
            bc_t = load(gl.tile([2, 1], F32, tag="bct", name="bct"), bc)
            cntbc_t = load(gl.tile([2, NGRAPH], F32, tag="cntbc",
                                   name="cntbc"), cntbc)
            spidx_t = load(gl.tile([128, NSPILL], I32, tag="spidx",
                                   name="spidx"), spill_idx)
            zb_t = gl.tile([128, 1], F32, tag="zb", name="zb")
            nc.vector.memset(zb_t[:], 0.0)

            u2_self = [gl.tile([128, WH, 32], BF16, tag=f"u2s{h}",
                               name=f"u2s{h}") for h in range(2)]
            v2_t = gl.tile([128, NBLK, 32], BF16, tag="v2t", name="v2t")
            stag = gl.tile([128, NBLK, 32], BF16, tag="stag", name="stag")

            # persistent pooling accumulator: [32 ch, 512 graphs] PSUM
            pool_ps = plp.tile([32, NGRAPH], F32, tag="poolps", name="poolps")

            # ---------------- layer 1 (host-staged, narrow slot scatter)
            with tc.tile_pool(name="L1", bufs=2) as lp1, \
                 tc.tile_pool(name="L1g", bufs=2) as gp1, \
                 tc.tile_pool(name="L1s", bufs=2) as sp1, \
                 tc.tile_pool(name="L1d", bufs=1) as dp1:

                dinv32_t = load(dp1.tile([32, WPC, 128], BF16, tag="d32", name="d32"),
                                dinv32, nc.sync)

                def fin1(w, y_ps):
                    h = w // WH
                    z1r = lp1.tile([128, 8], BF16, tag="z1r")
                    nc.vector.tensor_scalar(
                        out=z1r[:], in0=y_ps[:],
                        scalar1=dinv_t[:, w:w + 1], scalar2=None,
                        op0=mybir.AluOpType.mult)
                    z1T_ps = tp.tile([8, 128], BF16, tag="trp")
                    nc.tensor.transpose(out=z1T_ps[:], in_=z1r[:],
                                        identity=eyeb_t[:])
                    z1T = lp1.tile([8, 128], BF16, tag="z1T")
                    nc.vector.tensor_copy(out=z1T[:], in_=z1T_ps[:])
                    h1ps = hp.tile([32, 128], F32, tag="hps")
                    nc.tensor.matmul(out=h1ps[:], lhsT=w1_t[:], rhs=z1T[:],
                                     start=True, stop=True,
                                     skip_group_check=True)
                    h1T = lp1.tile([32, 128], BF16, tag="h1T")
                    nc.scalar.activation(out=h1T[:], in_=h1ps[:],
                                         func=AF.Relu, bias=b1_t[:],
                                         scale=1.0)
                    v2T_ps = hp.tile([32, 128], F32, tag="hps")
                    nc.tensor.matmul(out=v2T_ps[:], lhsT=w2_t[:], rhs=h1T[:],
                                     start=True, stop=True,
                                     skip_group_check=True)
                    v2T = lp1.tile([32, 128], BF16, tag="v2T")
                    nc.vector.tensor_tensor(
                        out=v2T[:], in0=v2T_ps[:], in1=dinv32_t[:, w, :],
                        op=mybir.AluOpType.mult)
                    v2r_ps = tp.tile([128, 32], BF16, tag="trp")
                    nc.tensor.transpose(out=v2r_ps[:], in_=v2T[:],
                                        identity=eyeb_t[:32, :32])
                    nc.vector.tensor_copy(
                        out=u2_self[h][:, w - h * WH, :], in_=v2r_ps[:])

                for w0 in range(0, WPC, WB1):
                    nw = min(WB1, WPC - w0)
                    a0, a1 = CH_OFF1[w0], CH_OFF1[w0 + nw]
                    G = gp1.tile([128, MAXB1, 8], BF16, tag="g1")
                    nc.sync.dma_start(out=G[:, :a1 - a0, :],
                                      in_=st1[:, a0:a1, :])
                    S = sp1.tile([128, MAXB1, 32], FP8, tag="sd1")
                    nc.scalar.dma_start(out=S[:, :a1 - a0, :],
                                        in_=s_dst1[:, a0:a1, :])
                    for wi in range(nw):
                        w = w0 + wi
                        y_ps = yp.tile([128, 8], F32, tag="yps")
                        coff = CH_OFF1[w] - a0
                        for j, (slot, first, last) in enumerate(CHUNKS1[w]):
                            k = coff + j
                            nc.tensor.matmul(
                                out=y_ps[32 * slot:32 * (slot + 1), :],
                                lhsT=S[:, k, :], rhs=G[:, k, :],
                                start=first, stop=last,
                                skip_group_check=True,
                                tile_position=(0, 32 * slot))
                        fin1(w, y_ps)

                # allgather halves
                for h in range(2):
                    nc.sync.dma_start(
                        out=v2_loc[h][:].rearrange("(w p) c -> p w c", p=128),
                        in_=u2_self[h][:])
                    nc.gpsimd.collective_compute(
                        "AllGather", mybir.AluOpType.bypass,
                        replica_groups=groups,
                        ins=[v2_loc[h].ap().opt()],
                        outs=[v2_gh[h].ap().opt()])

            # assemble v2 table (SBUF, block-major) + v2_dram (for spills)
            for h in range(2):
                for cc in range(NCORES):
                    nc.sync.dma_start(
                        out=v2_t[:, cc * WPC + h * WH:
                                 cc * WPC + h * WH + WH, :],
                        in_=v2_gh[h][cc].rearrange("(w p) c -> p w c", p=128))
            nc.scalar.dma_start(
                out=v2_dram[:].rearrange("(p b) c -> p b c", p=128),
                in_=v2_t[:])

            # ---------------- layer 2
            with tc.tile_pool(name="L2", bufs=2) as lp2, \
                 tc.tile_pool(name="L2g", bufs=2) as g2p, \
                 tc.tile_pool(name="L2s", bufs=2) as sp2, \
                 tc.tile_pool(name="L2ss", bufs=2) as ssp2, \
                 tc.tile_pool(name="L2sp", bufs=1) as spp:

                # spill row-gathers (batched indirect)
                spillG = spp.tile([128, NSPILL, 32], BF16, tag="spillG", name="spillG")
                for t in range(NSPILL):
                    nc.gpsimd.indirect_dma_start(
                        out=spillG[:, t, :], out_offset=None,
                        in_=v2_dram[:],
                        in_offset=bass.IndirectOffsetOnAxis(
                            ap=spidx_t[:, t:t + 1], axis=0))

                def spill_win(w, y_ps, first):
                    t0, t1 = int(SPILL_T0[w]), int(SPILL_T0[w + 1])
                    ssp = lp2.tile([128, 36, 128], FP8, tag="sspill")
                    enge = nc.sync if w % 2 == 0 else nc.scalar
                    enge.dma_start(out=ssp[:, :t1 - t0, :],
                                   in_=s_spill[:, t0:t1, :])
                    for k in range(t1 - t0):
                        nc.tensor.matmul(
                            out=y_ps[:], lhsT=ssp[:, k, :],
                            rhs=spillG[:, t0 + k, :],
                            start=(first and k == 0),
                            stop=(k == t1 - t0 - 1),
                            skip_group_check=True)

                def fin2(w, y_ps):
                    h2pre = lp2.tile([128, 32], F32, tag="h2pre")
                    nc.vector.tensor_scalar(
                        out=h2pre[:], in0=y_ps[:],
                        scalar1=dinv_t[:, w:w + 1], scalar2=None,
                        op0=mybir.AluOpType.mult)
                    h2a = lp2.tile([128, 32], F32, tag="h2a")
                    nc.vector.tensor_tensor(
                        out=h2a[:], in0=h2pre[:], in1=b2bc_t[:],
                        op=mybir.AluOpType.add)
                    h2row = lp2.tile([128, 32], BF16, tag="h2row")
                    nc.scalar.activation(out=h2row[:], in_=h2a[:],
                                         func=AF.Relu, bias=zb_t[:],
                                         scale=1.0)
                    # pooling: h2row stationary, fat one-hot moving
                    spw = lp2.tile([128, NGRAPH], FP8, tag="spw")
                    nc.gpsimd.dma_start(out=spw[:], in_=s_pool[:, w, :])
                    nc.tensor.matmul(
                        out=pool_ps[:], lhsT=h2row[:], rhs=spw[:],
                        start=(w == 0), stop=(w == WPC - 1),
                        skip_group_check=True)

                for g in range(NWG):
                    # gather: one-hot matmuls -> stag (SBUF)
                    for ch0 in range(0, NBLK, 32):
                        nb = min(32, NBLK - ch0)
                        ssrc = ssp2.tile([128, 32, 128], FP8, tag="ssrc")
                        eng = nc.sync if (ch0 // 32) % 2 == 0 else nc.scalar
                        eng.dma_start(
                            out=ssrc[:, :nb, :],
                            in_=s_src[:, g * NBLK + ch0:
                                      g * NBLK + ch0 + nb, :])
                        for h16 in range(0, nb, 16):
                            nbb = min(16, nb - h16)
                            gbank = gbp.tile([128, 16, 32], F32, tag="gbank")
                            for i in range(nbb):
                                Ip = ch0 + h16 + i
                                I = (Ip % 49) * 16 + Ip // 49
                                nc.tensor.matmul(
                                    out=gbank[:, i, :],
                                    lhsT=ssrc[:, h16 + i, :],
                                    rhs=v2_t[:, I, :], start=True, stop=True,
                                    skip_group_check=True)
                            nc.vector.tensor_copy(
                                out=stag[:, ch0 + h16:ch0 + h16 + nbb, :],
                                in_=gbank[:, :nbb, :])

                    for w_lo in range(16):
                        w = g * 16 + w_lo
                        # permute: one SBUF->SBUF DMA (128 descs x 3136 B)
                        G2 = g2p.tile([128, 49, 32], BF16, tag="g2")
                        engp = nc.scalar if w_lo % 2 == 0 else nc.sync
                        engp.dma_start(
                            out=G2[:, :, :],
                            in_=stag[8 * w_lo:8 * w_lo + 8, :, :])
                        if w_lo % 2 == 0:
                            sd2 = sp2.tile([128, 98, 128], FP8, tag="sd2")
                            nwin = 2 if w + 1 < 96 else 1
                            eng = nc.sync if w_lo % 4 == 0 else nc.scalar
                            eng.dma_start(
                                out=sd2[:, :49 * nwin, :],
                                in_=s_dst2[:, w * 49:(w + nwin) * 49, :])
                        y_ps = yp.tile([128, 32], F32, tag="yps")
                        so = 49 * (w_lo % 2)
                        for t in range(49):
                            nc.tensor.matmul(
                                out=y_ps[:], lhsT=sd2[:, so + t, :],
                                rhs=G2[:, t, :],
                                start=(t == 0), stop=False,
                                skip_group_check=True)
                        spill_win(w, y_ps, False)
                        fin2(w, y_ps)
                for w in (96, 97):
                    y_ps = yp.tile([128, 32], F32, tag="yps")
                    spill_win(w, y_ps, True)
                    fin2(w, y_ps)

            # ---------------- head (transpose-free)
            with tc.tile_pool(name="head", bufs=1) as hd:
                pool_acc = hd.tile([32, NGRAPH], F32, tag="poolacc")
                nc.vector.tensor_copy(out=pool_acc[:], in_=pool_ps[:])
                nc.sync.dma_start(out=pool_in[:], in_=pool_acc[:])
                nc.gpsimd.collective_compute(
                    "AllReduce", mybir.AluOpType.add, replica_groups=groups,
                    ins=[pool_in.ap().opt()], outs=[pool_out.ap().opt()])
                psb = hd.tile([32, NGRAPH], F32, tag="psb")
                nc.sync.dma_start(out=psb[:], in_=pool_out[:])
                res_ps = hp.tile([2, NGRAPH], F32, tag="hps")
                nc.tensor.matmul(out=res_ps[:], lhsT=wc_t[:], rhs=psb[:],
                                 start=True, stop=True,
                                 skip_group_check=True)
                res1 = hd.tile([2, NGRAPH], F32, tag="res1")
                nc.vector.tensor_tensor(
                    out=res1[:], in0=res_ps[:], in1=cntbc_t[:],
                    op=mybir.AluOpType.mult)
                res = hd.tile([2, NGRAPH], F32, tag="res")
                nc.vector.tensor_scalar(
                    out=res[:], in0=res1[:], scalar1=bc_t[:],
                    scalar2=None, op0=mybir.AluOpType.add)
                for g in range(4):
                    for k in range(2):
                        nc.sync.dma_start(
                            out=out[g * 128:(g + 1) * 128, k:k + 1],
                            in_=res[k:k + 1, g * 128:(g + 1) * 128])
    return nc


# ------------------------------------------------------------------- runner
def kernel(**inputs):
    global _LAST_RESULTS
    import os

    _install_patches()
    from concourse.bass_utils import run_bass_kernel_spmd

    per_core, meta = _prepare(**inputs)
    nc = _build_nc(meta)
    trace = bool(os.environ.get("GCN_TRACE"))
    kw = {}
    if trace:
        kw = dict(trace=True, trace_cores=[0, 3])
    res = run_bass_kernel_spmd(
        nc, per_core, core_ids=list(range(NCORES)), **kw)
    _LAST_RESULTS = res
    return np.asarray(res.results[0]["out"], dtype=np.float32)
